# revision 1
# baseline (speedup 1.0000x reference)
"""Linear attention (elu(x)+1 feature map) Bass/Tile kernel for Trainium2.

Problem: B=4, H=16, S=4096, D=64, fp32.
  Qf = elu(Q)+1; Kf = (elu(K)+1)*mask
  KV = einsum('bhsd,bhse->bhde', Kf, V); Ksum = sum_s Kf
  out = (Qf @ KV) / (Qf . Ksum)

Sharding: the 64 (b,h) pairs are data-parallel; each of the 8 cores gets 8
pairs. No collectives.

Per-core layout (pairs processed in 4 groups of 2, "A"/"B"):
  phase A (per 512-row sub-tile): DMA Q/K/V/mask; Qf/Kf = min(exp(x),1)+relu(x)
  (exact identity for elu(x)+1); V *= mask (broadcast). Accumulate
  [KV|Ksum] = Kf_chunk^T @ [V*m|m] in PSUM over 32 chunks of 128 rows.
  Qf chunks are PE-transposed (pairs A+B interleaved -> full 128-partition
  tiles) into Qt[128=(pair,d), 4096].
  phase B: one 128x130 matmul per chunk with block-diag [[KVK_A,0],[0,KVK_B]]
  gives [outA|ZnumA|outB|ZnumB] in natural [s,d] layout; normalize via
  reciprocal + ACT copy-with-scale; contiguous 32KB output DMAs.
"""

import numpy as np

import concourse.bass as bass
import concourse.mybir as mybir
import concourse.tile as tile
from concourse.bass_utils import run_bass_kernel_spmd
from concourse.masks import make_identity

F32 = mybir.dt.float32
AF = mybir.ActivationFunctionType

N_CORES = 8
PAIRS = 8          # (b,h) pairs per core
S = 4096
D = 64
SUBS = 4           # sub-tiles per pair
CHUNKS_PER_SUB = 8  # 128-row chunks per sub-tile
CHUNKS = SUBS * CHUNKS_PER_SUB  # 32


def build_bass() -> bass.Bass:
    from concourse.bacc import Bacc
    nc = Bacc()
    Qh = nc.dram_tensor("Q", [PAIRS, S, D], F32, kind="ExternalInput")
    Kh = nc.dram_tensor("K", [PAIRS, S, D], F32, kind="ExternalInput")
    Vh = nc.dram_tensor("V", [PAIRS, S, D], F32, kind="ExternalInput")
    Mh = nc.dram_tensor("mask", [PAIRS, S], F32, kind="ExternalOutput" if False else "ExternalInput")
    Oh = nc.dram_tensor("O", [PAIRS, S, D], F32, kind="ExternalOutput")

    # DRAM views per pair-group g (pairs 2g, 2g+1):
    # [SUBS, 128, CHUNKS_PER_SUB, 2, D] with s = t*512 + c*128 + p, u = pair
    def gview(h, g):
        return h[2 * g:2 * g + 2].rearrange(
            "u (t c p) d -> t p c u d", t=SUBS, c=CHUNKS_PER_SUB, p=128)

    Qv = [gview(Qh, g) for g in range(PAIRS // 2)]
    Kv = [gview(Kh, g) for g in range(PAIRS // 2)]
    Vv = [gview(Vh, g) for g in range(PAIRS // 2)]
    Mv = [Mh[2 * g:2 * g + 2].rearrange(
        "u (t c p) -> t p c u", t=SUBS, c=CHUNKS_PER_SUB, p=128)
        for g in range(PAIRS // 2)]

    with tile.TileContext(nc) as tc:
        from contextlib import ExitStack
        with ExitStack() as ctx:
            consts = ctx.enter_context(tc.tile_pool(name="consts", bufs=1))
            qr_pool = ctx.enter_context(tc.tile_pool(name="qr", bufs=4))
            kr_pool = ctx.enter_context(tc.tile_pool(name="kr", bufs=4))
            vm_pool = ctx.enter_context(tc.tile_pool(name="vm", bufs=4))
            qf_pool = ctx.enter_context(tc.tile_pool(name="qf", bufs=4))
            kf_pool = ctx.enter_context(tc.tile_pool(name="kf", bufs=4))
            qt_pool = ctx.enter_context(tc.tile_pool(name="qt", bufs=2))
            bd_pool = ctx.enter_context(tc.tile_pool(name="bd", bufs=2))
            osb_pool = ctx.enter_context(tc.tile_pool(name="osb", bufs=6))
            rec_pool = ctx.enter_context(tc.tile_pool(name="rec", bufs=6))
            kv_psum = ctx.enter_context(tc.tile_pool(name="kvps", bufs=4, space="PSUM"))
            tp_psum = ctx.enter_context(tc.tile_pool(name="tpps", bufs=2, space="PSUM"))
            ob_psum = ctx.enter_context(tc.tile_pool(name="obps", bufs=2, space="PSUM"))

            identity = consts.tile([128, 128], F32)
            make_identity(nc, identity)

            for g in range(PAIRS // 2):
                pA, pB = 2 * g, 2 * g + 1
                kv_ps = [kv_psum.tile([64, 65], F32, tag="kv", name=f"kv_{g}_0"),
                         kv_psum.tile([64, 65], F32, tag="kv", name=f"kv_{g}_1")]
                qt = qt_pool.tile([128, CHUNKS, 128], F32)

                for t in range(SUBS):
                    qr = qr_pool.tile([128, CHUNKS_PER_SUB, 2, D], F32)
                    kr = kr_pool.tile([128, CHUNKS_PER_SUB, 2, D], F32)
                    vm = vm_pool.tile([128, CHUNKS_PER_SUB, 2, D + 1], F32)
                    for pi in range(2):
                        nc.sync.dma_start(out=qr[:, :, pi], in_=Qv[g][t][:, :, pi])
                        nc.sync.dma_start(out=kr[:, :, pi], in_=Kv[g][t][:, :, pi])
                        nc.sync.dma_start(out=vm[:, :, pi, 0:D], in_=Vv[g][t][:, :, pi])
                        nc.sync.dma_start(out=vm[:, :, pi, D], in_=Mv[g][t][:, :, pi])

                    qf = qf_pool.tile([128, CHUNKS_PER_SUB, 2, D], F32)
                    kf = kf_pool.tile([128, CHUNKS_PER_SUB, 2, D], F32)
                    # elu(x)+1 == min(exp(x),1) + relu(x)
                    nc.scalar.activation(qf, qr, AF.Exp)
                    nc.vector.tensor_scalar_min(qf, qf, 1.0)
                    nc.scalar.activation(qr, qr, AF.Relu)
                    nc.vector.tensor_add(qf, qf, qr)
                    nc.scalar.activation(kf, kr, AF.Exp)
                    nc.vector.tensor_scalar_min(kf, kf, 1.0)
                    nc.scalar.activation(kr, kr, AF.Relu)
                    nc.vector.tensor_add(kf, kf, kr)
                    # V *= mask (mask column broadcast over D)
                    mcol = vm[:, :, :, D:D + 1].to_broadcast([128, CHUNKS_PER_SUB, 2, D])
                    nc.vector.tensor_tensor(
                        out=vm[:, :, :, 0:D], in0=vm[:, :, :, 0:D], in1=mcol,
                        op=mybir.AluOpType.mult)

                    for c in range(CHUNKS_PER_SUB):
                        cc = t * CHUNKS_PER_SUB + c
                        for pi in range(2):
                            nc.tensor.matmul(
                                kv_ps[pi], lhsT=kf[:, c, pi], rhs=vm[:, c, pi],
                                start=(cc == 0), stop=(cc == CHUNKS - 1))
                        tp = tp_psum.tile([128, 128], F32)
                        nc.tensor.transpose(
                            tp, qf[:, c].rearrange("p a d -> p (a d)"), identity)
                        nc.scalar.activation(qt[:, cc, :], tp, AF.Copy)

                # block-diagonal [[KVK_A, 0], [0, KVK_B]]
                bd = bd_pool.tile([128, 130], F32)
                nc.vector.memset(bd, 0.0)
                nc.vector.tensor_copy(bd[0:64, 0:65], kv_ps[0])
                nc.vector.tensor_copy(bd[64:128, 65:130], kv_ps[1])

                for cc in range(CHUNKS):
                    ob = ob_psum.tile([128, 130], F32)
                    nc.tensor.matmul(ob, lhsT=qt[:, cc, :], rhs=bd,
                                     start=True, stop=True)
                    rec = rec_pool.tile([128, 2], F32)
                    nc.vector.reciprocal(rec[:, 0:1], ob[:, 64:65])
                    nc.vector.reciprocal(rec[:, 1:2], ob[:, 129:130])
                    osb = osb_pool.tile([128, 2, D], F32)
                    nc.scalar.activation(osb[:, 0], ob[:, 0:64], AF.Copy,
                                         scale=rec[:, 0:1])
                    nc.scalar.activation(osb[:, 1], ob[:, 65:129], AF.Copy,
                                         scale=rec[:, 1:2])
                    nc.sync.dma_start(out=Oh[pA, bass.ts(cc, 128), :], in_=osb[:, 0])
                    nc.sync.dma_start(out=Oh[pB, bass.ts(cc, 128), :], in_=osb[:, 1])
    nc.finalize()
    return nc


_NC_CACHE = None


def _get_nc():
    global _NC_CACHE
    if _NC_CACHE is None:
        _NC_CACHE = build_bass()
    return _NC_CACHE


def kernel(Q: np.ndarray, K: np.ndarray, V: np.ndarray, mask: np.ndarray,
           _trace: bool = False):
    B, H = 4, 16
    NP = B * H
    per = NP // N_CORES
    Qr = np.ascontiguousarray(np.asarray(Q, dtype=np.float32).reshape(NP, S, D))
    Kr = np.ascontiguousarray(np.asarray(K, dtype=np.float32).reshape(NP, S, D))
    Vr = np.ascontiguousarray(np.asarray(V, dtype=np.float32).reshape(NP, S, D))
    Mr = np.ascontiguousarray(np.asarray(mask, dtype=np.float32).reshape(NP, S))

    in_maps = []
    for i in range(N_CORES):
        sl = slice(i * per, (i + 1) * per)
        in_maps.append({
            "Q": np.ascontiguousarray(Qr[sl]),
            "K": np.ascontiguousarray(Kr[sl]),
            "V": np.ascontiguousarray(Vr[sl]),
            "mask": np.ascontiguousarray(Mr[sl]),
        })

    nc = _get_nc()
    res = run_bass_kernel_spmd(nc, in_maps, core_ids=list(range(N_CORES)),
                               trace=_trace)
    out = np.concatenate([r["O"] for r in res.results], axis=0)
    if _trace:
        kernel._last_results = res
    return out.reshape(B, H, S, D)



# revision 5
# speedup vs baseline: 1.7695x; 1.7695x over previous
"""Linear attention (elu(x)+1 feature map) Bass/Tile kernel for Trainium2.

Problem: B=4, H=16, S=4096, D=64, fp32.
  Qf = elu(Q)+1; Kf = (elu(K)+1)*mask
  KV = einsum('bhsd,bhse->bhde', Kf, V); Ksum = sum_s Kf
  out = (Qf @ KV) / (Qf . Ksum)

Sharding: the 64 (b,h) pairs are data-parallel; each of the 8 cores gets 8
pairs. No collectives.

v2 design (vs the fp32 baseline at 383us):
  - all matmul operands in bf16 (PE 1 cyc/row instead of 4), PSUM stays fp32
  - feature map in 3 ops/tensor: exp (ACT), relu (ACT for Q / DVE for K),
    fused (min(e,1) + r) via scalar_tensor_tensor (DVE); ops span a
    half-group (16 chunks x 2 pairs, FD=2048) to amortize per-op overhead
  - one KV matmul per 128-row chunk covering both pairs of a group:
    lhsT=[kfA|kfB] (128 cols), rhs=[vmA|vmB] (130 cols); junk in the
    off-diagonal blocks is dropped when building the block-diag bd
  - bd layout [KV_A | KV_B | ksumA | ksumB] so phase-B output is
    [outA(64) | outB(64) | ZnumA | ZnumB]: batched reciprocal + one
    broadcast tensor_tensor multiply per 3-chunk PSUM bank
  - PE transposes write bf16 PSUM; PSUM->SBUF copies batched per 8 chunks
  - 512KB input DMAs, 1MB output DMAs
"""

import numpy as np

import concourse.bass as bass
import concourse.mybir as mybir
import concourse.tile as tile
from concourse.bass_utils import run_bass_kernel_spmd
from concourse.masks import make_identity

F32 = mybir.dt.float32
BF16 = mybir.dt.bfloat16
AF = mybir.ActivationFunctionType
ALU = mybir.AluOpType

N_CORES = 8
PAIRS = 8          # (b,h) pairs per core
S = 4096
D = 64
HALVES = 2         # half-groups per pair-group
CH = 16            # 128-row chunks per half-group
CHUNKS = HALVES * CH  # 32
TPB = 8            # transpose chunks batched per PSUM bank
OB_BATCH = 3       # phase-B chunks per PSUM bank (3*130*4B <= 2KB)


def build_bass() -> bass.Bass:
    from concourse.bacc import Bacc
    nc = Bacc()
    Qh = nc.dram_tensor("Q", [PAIRS, S, D], F32, kind="ExternalInput")
    Kh = nc.dram_tensor("K", [PAIRS, S, D], F32, kind="ExternalInput")
    Vh = nc.dram_tensor("V", [PAIRS, S, D], F32, kind="ExternalInput")
    Mh = nc.dram_tensor("mask", [PAIRS, S], F32, kind="ExternalInput")
    Oh = nc.dram_tensor("O", [PAIRS, S, D], F32, kind="ExternalOutput")

    # per-pair DRAM views; s = h*2048 + c*128 + p
    def pview(t, p):
        return t[p].rearrange("(h c p) d -> h p c d", h=HALVES, c=CH, p=128)

    Qv = [pview(Qh, p) for p in range(PAIRS)]
    Kv = [pview(Kh, p) for p in range(PAIRS)]
    Vv = [pview(Vh, p) for p in range(PAIRS)]
    Mv = [Mh[p].rearrange("(h c p) -> h p c", h=HALVES, c=CH, p=128)
          for p in range(PAIRS)]
    Ov = [Oh[p].rearrange("(c p) d -> p c d", p=128) for p in range(PAIRS)]

    with tile.TileContext(nc) as tc:
        from contextlib import ExitStack
        with ExitStack() as ctx:
            consts = ctx.enter_context(tc.tile_pool(name="consts", bufs=1))
            qr_pool = ctx.enter_context(tc.tile_pool(name="qr", bufs=2))
            kr_pool = ctx.enter_context(tc.tile_pool(name="kr", bufs=2))
            vr_pool = ctx.enter_context(tc.tile_pool(name="vr", bufs=2))
            mk_pool = ctx.enter_context(tc.tile_pool(name="mk", bufs=2))
            e_pool = ctx.enter_context(tc.tile_pool(name="e", bufs=2))
            r_pool = ctx.enter_context(tc.tile_pool(name="r", bufs=2))
            kf_pool = ctx.enter_context(tc.tile_pool(name="kf", bufs=2))
            vm_pool = ctx.enter_context(tc.tile_pool(name="vm", bufs=2))
            qf_pool = ctx.enter_context(tc.tile_pool(name="qf", bufs=2))
            qt_pool = ctx.enter_context(tc.tile_pool(name="qt", bufs=2))
            bd_pool = ctx.enter_context(tc.tile_pool(name="bd", bufs=2))
            osb_pool = ctx.enter_context(tc.tile_pool(name="osb", bufs=2))
            zr_pool = ctx.enter_context(tc.tile_pool(name="zr", bufs=3))
            kv_psum = ctx.enter_context(tc.tile_pool(name="kvps", bufs=2, space="PSUM"))
            tp_psum = ctx.enter_context(tc.tile_pool(name="tpps", bufs=2, space="PSUM"))
            ob_psum = ctx.enter_context(tc.tile_pool(name="obps", bufs=3, space="PSUM"))

            identity = consts.tile([128, 128], BF16)
            make_identity(nc, identity)

            for g in range(PAIRS // 2):
                pA, pB = 2 * g, 2 * g + 1
                kv_ps = kv_psum.tile([128, 130], F32, tag="kv")
                qf = qf_pool.tile([128, CHUNKS, 2, D], BF16, tag="qf")
                qt = qt_pool.tile([128, CHUNKS, 128], BF16, tag="qt")

                for h in range(HALVES):
                    qr = qr_pool.tile([128, CH, 2, D], F32, tag="qr")
                    kr = kr_pool.tile([128, CH, 2, D], F32, tag="kr")
                    vr = vr_pool.tile([128, CH, 2, D], F32, tag="vr")
                    mk = mk_pool.tile([128, CH, 2, 1], F32, tag="mk")
                    for u, p in ((0, pA), (1, pB)):
                        nc.sync.dma_start(out=qr[:, :, u], in_=Qv[p][h])
                        nc.sync.dma_start(out=kr[:, :, u], in_=Kv[p][h])
                        nc.sync.dma_start(out=vr[:, :, u], in_=Vv[p][h])
                        nc.sync.dma_start(out=mk[:, :, u, 0], in_=Mv[p][h])

                    # elu(x)+1 == min(exp(x),1) + relu(x)
                    eq = e_pool.tile([128, CH, 2, D], BF16, tag="eq")
                    ek = e_pool.tile([128, CH, 2, D], BF16, tag="ek")
                    rq = r_pool.tile([128, CH, 2, D], BF16, tag="rq")
                    rk = r_pool.tile([128, CH, 2, D], BF16, tag="rk")
                    kf = kf_pool.tile([128, CH, 2, D], BF16, tag="kf")
                    vm = vm_pool.tile([128, CH, 2, D + 1], BF16, tag="vm")

                    nc.scalar.activation(eq, qr, AF.Exp)
                    nc.scalar.activation(rq, qr, AF.Relu)
                    nc.vector.scalar_tensor_tensor(
                        out=qf[:, bass.ts(h, CH)], in0=eq, scalar=1.0,
                        in1=rq, op0=ALU.min, op1=ALU.add)
                    nc.scalar.activation(ek, kr, AF.Exp)
                    nc.vector.tensor_scalar_max(rk, kr, 0.0)
                    nc.vector.scalar_tensor_tensor(
                        out=kf, in0=ek, scalar=1.0, in1=rk,
                        op0=ALU.min, op1=ALU.add)
                    # vm = [V * mask | mask]
                    nc.vector.tensor_tensor(
                        out=vm[:, :, :, 0:D], in0=vr,
                        in1=mk.to_broadcast([128, CH, 2, D]),
                        op=ALU.mult)
                    nc.vector.tensor_copy(vm[:, :, :, D:D + 1], mk)

                    for b in range(CH // TPB):
                        tp = tp_psum.tile([128, TPB, 128], BF16, tag="tp")
                        for c in range(TPB):
                            cl = b * TPB + c
                            cc = h * CH + cl
                            # [kfA|kfB]^T @ [vmA|vmB]: diag blocks are KV_A/KV_B
                            nc.tensor.matmul(
                                kv_ps,
                                lhsT=kf[:, cl].rearrange("p u d -> p (u d)"),
                                rhs=vm[:, cl].rearrange("p u e -> p (u e)"),
                                start=(cc == 0), stop=(cc == CHUNKS - 1))
                            nc.tensor.transpose(
                                tp[:, c],
                                qf[:, cc].rearrange("p u d -> p (u d)"),
                                identity)
                        nc.vector.tensor_copy(
                            qt[:, h * CH + b * TPB:h * CH + (b + 1) * TPB], tp)

                # bd = [[KV_A, 0, ksumA, 0], [0, KV_B, 0, ksumB]] (128x130)
                bd = bd_pool.tile([128, 130], BF16, tag="bd")
                nc.gpsimd.memset(bd, 0.0)
                nc.vector.tensor_copy(bd[0:64, 0:64], kv_ps[0:64, 0:64])
                nc.vector.tensor_copy(bd[64:128, 64:128], kv_ps[64:128, 65:129])
                nc.vector.tensor_copy(bd[0:64, 128:129], kv_ps[0:64, 64:65])
                nc.vector.tensor_copy(bd[64:128, 129:130], kv_ps[64:128, 129:130])

                out_sb = osb_pool.tile([128, CHUNKS, 2, D], F32, tag="osb")
                for j in range((CHUNKS + OB_BATCH - 1) // OB_BATCH):
                    c0 = j * OB_BATCH
                    n = min(OB_BATCH, CHUNKS - c0)
                    ob = ob_psum.tile([128, OB_BATCH, 130], F32, tag="ob")
                    for k in range(n):
                        nc.tensor.matmul(ob[:, k], lhsT=qt[:, c0 + k], rhs=bd,
                                         start=True, stop=True)
                    zr = zr_pool.tile([128, OB_BATCH, 2, 1], F32, tag="zr")
                    nc.vector.reciprocal(zr[:, 0:n, :, 0], ob[:, 0:n, 128:130])
                    nc.vector.tensor_tensor(
                        out=out_sb[:, c0:c0 + n],
                        in0=ob[:, 0:n, 0:128].rearrange(
                            "p c (u e) -> p c u e", u=2),
                        in1=zr[:, 0:n].to_broadcast([128, n, 2, D]),
                        op=ALU.mult)
                for u, p in ((0, pA), (1, pB)):
                    nc.sync.dma_start(out=Ov[p], in_=out_sb[:, :, u])
    nc.finalize()
    return nc


_NC_CACHE = None


def _get_nc():
    global _NC_CACHE
    if _NC_CACHE is None:
        _NC_CACHE = build_bass()
    return _NC_CACHE


def kernel(Q: np.ndarray, K: np.ndarray, V: np.ndarray, mask: np.ndarray,
           _trace: bool = False):
    B, H = 4, 16
    NP = B * H
    per = NP // N_CORES
    Qr = np.ascontiguousarray(np.asarray(Q, dtype=np.float32).reshape(NP, S, D))
    Kr = np.ascontiguousarray(np.asarray(K, dtype=np.float32).reshape(NP, S, D))
    Vr = np.ascontiguousarray(np.asarray(V, dtype=np.float32).reshape(NP, S, D))
    Mr = np.ascontiguousarray(np.asarray(mask, dtype=np.float32).reshape(NP, S))

    in_maps = []
    for i in range(N_CORES):
        sl = slice(i * per, (i + 1) * per)
        in_maps.append({
            "Q": np.ascontiguousarray(Qr[sl]),
            "K": np.ascontiguousarray(Kr[sl]),
            "V": np.ascontiguousarray(Vr[sl]),
            "mask": np.ascontiguousarray(Mr[sl]),
        })

    nc = _get_nc()
    res = run_bass_kernel_spmd(nc, in_maps, core_ids=list(range(N_CORES)),
                               trace=_trace)
    out = np.concatenate([r["O"] for r in res.results], axis=0)
    if _trace:
        kernel._last_results = res
    return out.reshape(B, H, S, D)


# revision 6
# speedup vs baseline: 2.7474x; 1.5527x over previous
"""Linear attention (elu(x)+1 feature map) Bass/Tile kernel for Trainium2.

Problem: B=4, H=16, S=4096, D=64, fp32.
  Qf = elu(Q)+1; Kf = (elu(K)+1)*mask
  KV = einsum('bhsd,bhse->bhde', Kf, V); Ksum = sum_s Kf
  out = (Qf @ KV) / (Qf . Ksum)

Sharding: the 64 (b,h) pairs are data-parallel; each of the 8 cores gets 8
pairs. No collectives.

v3 design notes:
  - sequence rows mapped s = p*32 + j (p = partition, j = chunk): every
    HBM<->SBUF DMA moves 4-8KB contiguous runs per partition (128
    descriptors instead of 2048) -- the v2 trace showed SP descriptor
    generation (2.2us/DMA) and DMA descriptor floor dominating.
    KV/Ksum are sums over s, and out rows follow the same relabeling,
    so the chunk remap is exact.
  - all matmul operands bf16 (PE 1 cyc/row), PSUM fp32
  - raw Q/K fp32 tiles are pair-major (contiguous DMA); V and mask are
    cast to bf16 during SWDGE DMA (gpsimd); elementwise ops bridge to
    chunk-major bf16 tiles via strided APs (innermost dim stays dense)
  - feature map: exp (ACT), relu (ACT for Q, DVE for K), min via
    tensor_scalar (4x bf16), add via tensor_tensor (2x bf16)
  - one KV matmul per chunk covers both pairs: lhsT=[kfA|kfB],
    rhs=[vmA|vmB] (130 cols: V*mask plus mask column for Ksum)
  - bd = [[KV_A,0,ksumA,0],[0,KV_B,0,ksumB]] so phase-B output is
    [outA|outB|ZnumA|ZnumB]: per 3-chunk PSUM bank one reciprocal +
    one broadcast multiply
  - PE transposes write bf16 PSUM, copies batched 8 chunks, alternating
    DVE/ACT
"""

import numpy as np

import concourse.bass as bass
import concourse.mybir as mybir
import concourse.tile as tile
from concourse.bass_utils import run_bass_kernel_spmd
from concourse.masks import make_identity

F32 = mybir.dt.float32
BF16 = mybir.dt.bfloat16
AF = mybir.ActivationFunctionType
ALU = mybir.AluOpType

N_CORES = 8
PAIRS = 8          # (b,h) pairs per core
S = 4096
D = 64
HALVES = 2         # half-groups (DMA/elementwise granularity)
CH = 16            # chunks per half-group
CHUNKS = HALVES * CH  # 32
TPB = 8            # transpose chunks batched per PSUM bank
OB_BATCH = 3       # phase-B chunks per PSUM bank (3*130*4B <= 2KB)


def build_bass() -> bass.Bass:
    from concourse.bacc import Bacc
    nc = Bacc()
    Qh = nc.dram_tensor("Q", [PAIRS, S, D], F32, kind="ExternalInput")
    Kh = nc.dram_tensor("K", [PAIRS, S, D], F32, kind="ExternalInput")
    Vh = nc.dram_tensor("V", [PAIRS, S, D], F32, kind="ExternalInput")
    Mh = nc.dram_tensor("mask", [PAIRS, S], F32, kind="ExternalInput")
    Oh = nc.dram_tensor("O", [PAIRS, S, D], F32, kind="ExternalOutput")

    # s = p*32 + j: partition-contiguous rows; [128, 2048] per pair
    Qv = [Qh[p].rearrange("(a j) d -> a (j d)", a=128) for p in range(PAIRS)]
    Kv = [Kh[p].rearrange("(a j) d -> a (j d)", a=128) for p in range(PAIRS)]
    Vv = [Vh[p].rearrange("(a j) d -> a (j d)", a=128) for p in range(PAIRS)]
    Mv = [Mh[p].rearrange("(a j) -> a j", a=128) for p in range(PAIRS)]
    Ov = [Oh[p].rearrange("(a j) d -> a (j d)", a=128) for p in range(PAIRS)]
    HB = CH * D  # elements per half per partition

    with tile.TileContext(nc) as tc:
        from contextlib import ExitStack
        with ExitStack() as ctx:
            consts = ctx.enter_context(tc.tile_pool(name="consts", bufs=1))
            qr_pool = ctx.enter_context(tc.tile_pool(name="qr", bufs=2))
            kr_pool = ctx.enter_context(tc.tile_pool(name="kr", bufs=2))
            vb_pool = ctx.enter_context(tc.tile_pool(name="vb", bufs=2))
            mb_pool = ctx.enter_context(tc.tile_pool(name="mb", bufs=2))
            e_pool = ctx.enter_context(tc.tile_pool(name="e", bufs=2))
            r_pool = ctx.enter_context(tc.tile_pool(name="r", bufs=2))
            kf_pool = ctx.enter_context(tc.tile_pool(name="kf", bufs=2))
            vm_pool = ctx.enter_context(tc.tile_pool(name="vm", bufs=2))
            qf_pool = ctx.enter_context(tc.tile_pool(name="qf", bufs=2))
            qt_pool = ctx.enter_context(tc.tile_pool(name="qt", bufs=2))
            bd_pool = ctx.enter_context(tc.tile_pool(name="bd", bufs=1))
            osb_pool = ctx.enter_context(tc.tile_pool(name="osb", bufs=2))
            zr_pool = ctx.enter_context(tc.tile_pool(name="zr", bufs=3))
            kv_psum = ctx.enter_context(tc.tile_pool(name="kvps", bufs=2, space="PSUM"))
            tp_psum = ctx.enter_context(tc.tile_pool(name="tpps", bufs=2, space="PSUM"))
            ob_psum = ctx.enter_context(tc.tile_pool(name="obps", bufs=3, space="PSUM"))

            identity = consts.tile([128, 128], BF16)
            make_identity(nc, identity)
            # bd zero regions never change: two preset buffers reused by
            # alternating groups
            bds = [consts.tile([128, 130], BF16, tag=f"bd{i}", name=f"bd{i}")
                   for i in range(2)]
            for b in bds:
                nc.gpsimd.memset(b, 0.0)

            for g in range(PAIRS // 2):
                pA, pB = 2 * g, 2 * g + 1
                kv_ps = kv_psum.tile([128, 130], F32, tag="kv")
                qf = qf_pool.tile([128, CHUNKS, 2, D], BF16, tag="qf")
                qt = qt_pool.tile([128, CHUNKS, 128], BF16, tag="qt")
                mb = mb_pool.tile([128, 2, CHUNKS, 1], BF16, tag="mb")
                for u, p in ((0, pA), (1, pB)):
                    nc.gpsimd.dma_start(out=mb[:, u, :, 0], in_=Mv[p])

                for h in range(HALVES):
                    qr = qr_pool.tile([128, 2, CH, D], F32, tag="qr")
                    kr = kr_pool.tile([128, 2, CH, D], F32, tag="kr")
                    vb = vb_pool.tile([128, 2, CH, D], BF16, tag="vb")
                    hs = slice(h * HB, (h + 1) * HB)
                    for u, p in ((0, pA), (1, pB)):
                        nc.sync.dma_start(
                            out=qr[:, u].rearrange("p j d -> p (j d)"),
                            in_=Qv[p][:, hs])
                        nc.sync.dma_start(
                            out=kr[:, u].rearrange("p j d -> p (j d)"),
                            in_=Kv[p][:, hs])
                        nc.gpsimd.dma_start(
                            out=vb[:, u].rearrange("p j d -> p (j d)"),
                            in_=Vv[p][:, hs])

                    # elu(x)+1 == min(exp(x),1) + relu(x)
                    eq = e_pool.tile([128, CH, 2, D], BF16, tag="eq")
                    ek = e_pool.tile([128, CH, 2, D], BF16, tag="ek")
                    rq = r_pool.tile([128, CH, 2, D], BF16, tag="rq")
                    rk = r_pool.tile([128, CH, 2, D], BF16, tag="rk")
                    kf = kf_pool.tile([128, CH, 2, D], BF16, tag="kf")
                    vm = vm_pool.tile([128, CH, 2, D + 1], BF16, tag="vm")

                    qrc = qr.rearrange("p u j d -> p j u d")
                    krc = kr.rearrange("p u j d -> p j u d")
                    nc.scalar.activation(eq, qrc, AF.Exp)
                    nc.scalar.activation(rq, qrc, AF.Relu)
                    nc.vector.tensor_scalar_min(eq, eq, 1.0)
                    nc.vector.tensor_add(qf[:, bass.ts(h, CH)], eq, rq)
                    nc.scalar.activation(ek, krc, AF.Exp)
                    nc.vector.tensor_scalar_max(rk, krc, 0.0)
                    nc.vector.tensor_scalar_min(ek, ek, 1.0)
                    nc.vector.tensor_add(kf, ek, rk)
                    # vm = [V * mask | mask]
                    mbh = mb[:, :, bass.ts(h, CH)].rearrange(
                        "p u j x -> p j u x")
                    nc.vector.tensor_tensor(
                        out=vm[:, :, :, 0:D],
                        in0=vb.rearrange("p u j d -> p j u d"),
                        in1=mbh.to_broadcast([128, CH, 2, D]),
                        op=ALU.mult)
                    nc.scalar.copy(vm[:, :, :, D:D + 1], mbh)

                    for b in range(CH // TPB):
                        tp = tp_psum.tile([128, TPB, 128], BF16, tag="tp")
                        for c in range(TPB):
                            cl = b * TPB + c
                            cc = h * CH + cl
                            # [kfA|kfB]^T @ [vmA|vmB]: diag blocks KV_A/KV_B
                            nc.tensor.matmul(
                                kv_ps,
                                lhsT=kf[:, cl].rearrange("p u d -> p (u d)"),
                                rhs=vm[:, cl].rearrange("p u e -> p (u e)"),
                                start=(cc == 0), stop=(cc == CHUNKS - 1))
                            nc.tensor.transpose(
                                tp[:, c],
                                qf[:, cc].rearrange("p u d -> p (u d)"),
                                identity)
                        qts = qt[:, h * CH + b * TPB:h * CH + (b + 1) * TPB]
                        if (h * 2 + b) % 2 == 0:
                            nc.vector.tensor_copy(qts, tp)
                        else:
                            nc.scalar.copy(qts, tp)

                # bd = [[KV_A, 0, ksumA, 0], [0, KV_B, 0, ksumB]] (128x130)
                bd = bds[g % 2]
                nc.scalar.copy(bd[0:64, 0:64], kv_ps[0:64, 0:64])
                nc.scalar.copy(bd[64:128, 64:128], kv_ps[64:128, 65:129])
                nc.scalar.copy(bd[0:64, 128:129], kv_ps[0:64, 64:65])
                nc.scalar.copy(bd[64:128, 129:130], kv_ps[64:128, 129:130])

                out_sb = osb_pool.tile([128, 2, CHUNKS, D], F32, tag="osb")
                for j in range((CHUNKS + OB_BATCH - 1) // OB_BATCH):
                    c0 = j * OB_BATCH
                    n = min(OB_BATCH, CHUNKS - c0)
                    ob = ob_psum.tile([128, OB_BATCH, 130], F32, tag="ob")
                    for k in range(n):
                        nc.tensor.matmul(ob[:, k], lhsT=qt[:, c0 + k], rhs=bd,
                                         start=True, stop=True)
                    zr = zr_pool.tile([128, OB_BATCH, 2, 1], F32, tag="zr")
                    nc.vector.reciprocal(zr[:, 0:n, :, 0], ob[:, 0:n, 128:130])
                    nc.vector.tensor_tensor(
                        out=out_sb[:, :, c0:c0 + n],
                        in0=ob[:, 0:n, 0:128].rearrange(
                            "p c (u e) -> p u c e", u=2),
                        in1=zr[:, 0:n].rearrange(
                            "p c u x -> p u c x").to_broadcast([128, 2, n, D]),
                        op=ALU.mult)
                for u, p in ((0, pA), (1, pB)):
                    nc.sync.dma_start(
                        out=Ov[p],
                        in_=out_sb[:, u].rearrange("p j d -> p (j d)"))
    nc.finalize()
    return nc


_NC_CACHE = None


def _get_nc():
    global _NC_CACHE
    if _NC_CACHE is None:
        _NC_CACHE = build_bass()
    return _NC_CACHE


def kernel(Q: np.ndarray, K: np.ndarray, V: np.ndarray, mask: np.ndarray,
           _trace: bool = False):
    B, H = 4, 16
    NP = B * H
    per = NP // N_CORES
    Qr = np.ascontiguousarray(np.asarray(Q, dtype=np.float32).reshape(NP, S, D))
    Kr = np.ascontiguousarray(np.asarray(K, dtype=np.float32).reshape(NP, S, D))
    Vr = np.ascontiguousarray(np.asarray(V, dtype=np.float32).reshape(NP, S, D))
    Mr = np.ascontiguousarray(np.asarray(mask, dtype=np.float32).reshape(NP, S))

    in_maps = []
    for i in range(N_CORES):
        sl = slice(i * per, (i + 1) * per)
        in_maps.append({
            "Q": np.ascontiguousarray(Qr[sl]),
            "K": np.ascontiguousarray(Kr[sl]),
            "V": np.ascontiguousarray(Vr[sl]),
            "mask": np.ascontiguousarray(Mr[sl]),
        })

    nc = _get_nc()
    res = run_bass_kernel_spmd(nc, in_maps, core_ids=list(range(N_CORES)),
                               trace=_trace)
    out = np.concatenate([r["O"] for r in res.results], axis=0)
    if _trace:
        kernel._last_results = res
    return out.reshape(B, H, S, D)


# revision 7
# speedup vs baseline: 2.8098x; 1.0227x over previous
"""Linear attention (elu(x)+1 feature map) Bass/Tile kernel for Trainium2.

Problem: B=4, H=16, S=4096, D=64, fp32.
  Qf = elu(Q)+1; Kf = (elu(K)+1)*mask
  KV = einsum('bhsd,bhse->bhde', Kf, V); Ksum = sum_s Kf
  out = (Qf @ KV) / (Qf . Ksum)

Sharding: the 64 (b,h) pairs are data-parallel; each of the 8 cores gets 8
pairs. No collectives.

v3 design notes:
  - sequence rows mapped s = p*32 + j (p = partition, j = chunk): every
    HBM<->SBUF DMA moves 4-8KB contiguous runs per partition (128
    descriptors instead of 2048) -- the v2 trace showed SP descriptor
    generation (2.2us/DMA) and DMA descriptor floor dominating.
    KV/Ksum are sums over s, and out rows follow the same relabeling,
    so the chunk remap is exact.
  - all matmul operands bf16 (PE 1 cyc/row), PSUM fp32
  - raw Q/K fp32 tiles are pair-major (contiguous DMA); V and mask are
    cast to bf16 during SWDGE DMA (gpsimd); elementwise ops bridge to
    chunk-major bf16 tiles via strided APs (innermost dim stays dense)
  - feature map: exp (ACT), relu (ACT for Q, DVE for K), min via
    tensor_scalar (4x bf16), add via tensor_tensor (2x bf16)
  - one KV matmul per chunk covers both pairs: lhsT=[kfA|kfB],
    rhs=[vmA|vmB] (130 cols: V*mask plus mask column for Ksum)
  - bd = [[KV_A,0,ksumA,0],[0,KV_B,0,ksumB]] so phase-B output is
    [outA|outB|ZnumA|ZnumB]: per 3-chunk PSUM bank one reciprocal +
    one broadcast multiply
  - PE transposes write bf16 PSUM, copies batched 8 chunks, alternating
    DVE/ACT
"""

import numpy as np

import concourse.bass as bass
import concourse.mybir as mybir
import concourse.tile as tile
from concourse.bass_utils import run_bass_kernel_spmd
from concourse.masks import make_identity

F32 = mybir.dt.float32
BF16 = mybir.dt.bfloat16
AF = mybir.ActivationFunctionType
ALU = mybir.AluOpType

N_CORES = 8
PAIRS = 8          # (b,h) pairs per core
S = 4096
D = 64
HALVES = 2         # half-groups (DMA/elementwise granularity)
CH = 16            # chunks per half-group
CHUNKS = HALVES * CH  # 32
TPB = 8            # transpose chunks batched per PSUM bank
OB_BATCH = 3       # phase-B chunks per PSUM bank (3*130*4B <= 2KB)


def build_bass() -> bass.Bass:
    from concourse.bacc import Bacc
    nc = Bacc()
    Qh = nc.dram_tensor("Q", [PAIRS, S, D], F32, kind="ExternalInput")
    Kh = nc.dram_tensor("K", [PAIRS, S, D], F32, kind="ExternalInput")
    Vh = nc.dram_tensor("V", [PAIRS, S, D], F32, kind="ExternalInput")
    Mh = nc.dram_tensor("mask", [PAIRS, S], F32, kind="ExternalInput")
    Oh = nc.dram_tensor("O", [PAIRS, S, D], F32, kind="ExternalOutput")

    # s = p*32 + j: partition-contiguous rows; [128, 2048] per pair
    Qv = [Qh[p].rearrange("(a j) d -> a (j d)", a=128) for p in range(PAIRS)]
    Kv = [Kh[p].rearrange("(a j) d -> a (j d)", a=128) for p in range(PAIRS)]
    Vv = [Vh[p].rearrange("(a j) d -> a (j d)", a=128) for p in range(PAIRS)]
    Mv = [Mh[p].rearrange("(a j) -> a j", a=128) for p in range(PAIRS)]
    Ov = [Oh[p].rearrange("(a j) d -> a (j d)", a=128) for p in range(PAIRS)]
    HB = CH * D  # elements per half per partition

    with tile.TileContext(nc) as tc:
        from contextlib import ExitStack
        with ExitStack() as ctx:
            consts = ctx.enter_context(tc.tile_pool(name="consts", bufs=1))
            qr_pool = ctx.enter_context(tc.tile_pool(name="qr", bufs=3))
            kr_pool = ctx.enter_context(tc.tile_pool(name="kr", bufs=3))
            vb_pool = ctx.enter_context(tc.tile_pool(name="vb", bufs=3))
            mb_pool = ctx.enter_context(tc.tile_pool(name="mb", bufs=2))
            e_pool = ctx.enter_context(tc.tile_pool(name="e", bufs=3))
            r_pool = ctx.enter_context(tc.tile_pool(name="r", bufs=3))
            kf_pool = ctx.enter_context(tc.tile_pool(name="kf", bufs=2))
            vm_pool = ctx.enter_context(tc.tile_pool(name="vm", bufs=2))
            qf_pool = ctx.enter_context(tc.tile_pool(name="qf", bufs=2))
            qt_pool = ctx.enter_context(tc.tile_pool(name="qt", bufs=2))
            bd_pool = ctx.enter_context(tc.tile_pool(name="bd", bufs=1))
            osb_pool = ctx.enter_context(tc.tile_pool(name="osb", bufs=2))
            zr_pool = ctx.enter_context(tc.tile_pool(name="zr", bufs=3))
            kv_psum = ctx.enter_context(tc.tile_pool(name="kvps", bufs=2, space="PSUM"))
            tp_psum = ctx.enter_context(tc.tile_pool(name="tpps", bufs=2, space="PSUM"))
            ob_psum = ctx.enter_context(tc.tile_pool(name="obps", bufs=3, space="PSUM"))

            identity = consts.tile([128, 128], BF16)
            make_identity(nc, identity)
            # bd zero regions never change: two preset buffers reused by
            # alternating groups
            bds = [consts.tile([128, 130], BF16, tag=f"bd{i}", name=f"bd{i}")
                   for i in range(2)]
            for b in bds:
                nc.gpsimd.memset(b, 0.0)

            for g in range(PAIRS // 2):
                pA, pB = 2 * g, 2 * g + 1
                kv_ps = kv_psum.tile([128, 132], F32, tag="kv")
                qf = qf_pool.tile([128, CHUNKS, 2, D], BF16, tag="qf")
                qt = qt_pool.tile([128, CHUNKS, 128], BF16, tag="qt")
                mb = mb_pool.tile([128, 2, CHUNKS, 1], BF16, tag="mb")
                for u, p in ((0, pA), (1, pB)):
                    nc.gpsimd.dma_start(out=mb[:, u, :, 0], in_=Mv[p])

                for h in range(HALVES):
                    qr = qr_pool.tile([128, 2, CH, D], F32, tag="qr")
                    kr = kr_pool.tile([128, 2, CH, D], F32, tag="kr")
                    vb = vb_pool.tile([128, 2, CH, D], BF16, tag="vb")
                    hs = slice(h * HB, (h + 1) * HB)
                    for u, p in ((0, pA), (1, pB)):
                        nc.sync.dma_start(
                            out=qr[:, u].rearrange("p j d -> p (j d)"),
                            in_=Qv[p][:, hs])
                        nc.sync.dma_start(
                            out=kr[:, u].rearrange("p j d -> p (j d)"),
                            in_=Kv[p][:, hs])
                        nc.gpsimd.dma_start(
                            out=vb[:, u].rearrange("p j d -> p (j d)"),
                            in_=Vv[p][:, hs])

                    # elu(x)+1 == min(exp(x),1) + relu(x)
                    eq = e_pool.tile([128, CH, 2, D], BF16, tag="eq")
                    ek = e_pool.tile([128, CH, 2, D], BF16, tag="ek")
                    rq = r_pool.tile([128, CH, 2, D], BF16, tag="rq")
                    rk = r_pool.tile([128, CH, 2, D], BF16, tag="rk")
                    kf = kf_pool.tile([128, CH, 2, D], BF16, tag="kf")
                    vm = vm_pool.tile([128, CH, 2, D + 2], BF16, tag="vm")

                    qrc = qr.rearrange("p u j d -> p j u d")
                    krc = kr.rearrange("p u j d -> p j u d")
                    nc.scalar.activation(eq, qrc, AF.Exp)
                    nc.scalar.activation(rq, qrc, AF.Relu)
                    nc.vector.tensor_scalar_min(eq, eq, 1.0)
                    nc.vector.tensor_add(qf[:, bass.ts(h, CH)], eq, rq)
                    nc.scalar.activation(ek, krc, AF.Exp)
                    nc.vector.tensor_scalar_max(rk, krc, 0.0)
                    nc.vector.tensor_scalar_min(ek, ek, 1.0)
                    nc.vector.tensor_add(kf, ek, rk)
                    # vm = [V * mask | mask]
                    mbh = mb[:, :, bass.ts(h, CH)].rearrange(
                        "p u j x -> p j u x")
                    nc.vector.tensor_tensor(
                        out=vm[:, :, :, 0:D],
                        in0=vb.rearrange("p u j d -> p j u d"),
                        in1=mbh.to_broadcast([128, CH, 2, D]),
                        op=ALU.mult)
                    nc.scalar.copy(vm[:, :, :, D:D + 1], mbh)

                    for b in range(CH // TPB):
                        tp = tp_psum.tile([128, TPB, 128], BF16, tag="tp")
                        for c in range(TPB):
                            cl = b * TPB + c
                            cc = h * CH + cl
                            # [kfA|kfB]^T @ [vmA|vmB]: diag blocks KV_A/KV_B
                            nc.tensor.matmul(
                                kv_ps,
                                lhsT=kf[:, cl].rearrange("p u d -> p (u d)"),
                                rhs=vm[:, cl].rearrange("p u e -> p (u e)"),
                                start=(cc == 0), stop=(cc == CHUNKS - 1))
                            nc.tensor.transpose(
                                tp[:, c],
                                qf[:, cc].rearrange("p u d -> p (u d)"),
                                identity)
                        qts = qt[:, h * CH + b * TPB:h * CH + (b + 1) * TPB]
                        if (h * 2 + b) % 2 == 0:
                            nc.vector.tensor_copy(qts, tp)
                        else:
                            nc.scalar.copy(qts, tp)

                # bd = [[KV_A, 0, ksumA, 0], [0, KV_B, 0, ksumB]] (128x130)
                bd = bds[g % 2]
                nc.scalar.copy(bd[0:64, 0:64], kv_ps[0:64, 0:64])
                nc.scalar.copy(bd[64:128, 64:128], kv_ps[64:128, 66:130])
                nc.scalar.copy(bd[0:64, 128:129], kv_ps[0:64, 64:65])
                nc.scalar.copy(bd[64:128, 129:130], kv_ps[64:128, 130:131])

                out_sb = osb_pool.tile([128, 2, CHUNKS, D], F32, tag="osb")
                for j in range((CHUNKS + OB_BATCH - 1) // OB_BATCH):
                    c0 = j * OB_BATCH
                    n = min(OB_BATCH, CHUNKS - c0)
                    ob = ob_psum.tile([128, OB_BATCH, 130], F32, tag="ob")
                    for k in range(n):
                        nc.tensor.matmul(ob[:, k], lhsT=qt[:, c0 + k], rhs=bd,
                                         start=True, stop=True)
                    zr = zr_pool.tile([128, OB_BATCH, 2, 1], F32, tag="zr")
                    nc.vector.reciprocal(zr[:, 0:n, :, 0], ob[:, 0:n, 128:130])
                    nc.vector.tensor_tensor(
                        out=out_sb[:, :, c0:c0 + n],
                        in0=ob[:, 0:n, 0:128].rearrange(
                            "p c (u e) -> p u c e", u=2),
                        in1=zr[:, 0:n].rearrange(
                            "p c u x -> p u c x").to_broadcast([128, 2, n, D]),
                        op=ALU.mult)
                for u, p in ((0, pA), (1, pB)):
                    nc.sync.dma_start(
                        out=Ov[p],
                        in_=out_sb[:, u].rearrange("p j d -> p (j d)"))
    nc.finalize()
    return nc


_NC_CACHE = None


def _get_nc():
    global _NC_CACHE
    if _NC_CACHE is None:
        _NC_CACHE = build_bass()
    return _NC_CACHE


def kernel(Q: np.ndarray, K: np.ndarray, V: np.ndarray, mask: np.ndarray,
           _trace: bool = False):
    B, H = 4, 16
    NP = B * H
    per = NP // N_CORES
    Qr = np.ascontiguousarray(np.asarray(Q, dtype=np.float32).reshape(NP, S, D))
    Kr = np.ascontiguousarray(np.asarray(K, dtype=np.float32).reshape(NP, S, D))
    Vr = np.ascontiguousarray(np.asarray(V, dtype=np.float32).reshape(NP, S, D))
    Mr = np.ascontiguousarray(np.asarray(mask, dtype=np.float32).reshape(NP, S))

    in_maps = []
    for i in range(N_CORES):
        sl = slice(i * per, (i + 1) * per)
        in_maps.append({
            "Q": np.ascontiguousarray(Qr[sl]),
            "K": np.ascontiguousarray(Kr[sl]),
            "V": np.ascontiguousarray(Vr[sl]),
            "mask": np.ascontiguousarray(Mr[sl]),
        })

    nc = _get_nc()
    res = run_bass_kernel_spmd(nc, in_maps, core_ids=list(range(N_CORES)),
                               trace=_trace)
    out = np.concatenate([r["O"] for r in res.results], axis=0)
    if _trace:
        kernel._last_results = res
    return out.reshape(B, H, S, D)
